# revision 8
# baseline (speedup 1.0000x reference)
"""Trainium2 Bass kernel for nn_Bsl2_9053791060551 (bi-GRU + segment reduce + MLP).

Self-contained: builds a Bass/Tile program per call and runs it SPMD on 8
NeuronCores, data-parallel over batch (8 sequences per core).

v2 architecture — chunked scan:
  - The GRU forgets its initial state in ~32 steps (measured 2.5e-7 @ W=32),
    so each direction's scan is split into C=16 time chunks of L=64 steps,
    run *as extra batch*: 16 chunks x 8 seqs = 128 moving columns per
    recurrent matmul (vs 8 in the naive layout). Each chunk is preceded by
    W=32 warmup steps (re-running the previous chunk's last 32 tokens from
    h=0); warmup output is discarded. Chunk 0 (fwd) / chunk 15 (bwd) reset
    h to the true zero initial state at the main-part boundary instead.
  - Scan runs D = L + W = 96 steps per direction; per step per dir:
    12 matmuls [128,128]@[128,128], gate math on [128,512]/[128,256] tiles
    spread across DVE / ACT / GPSIMD.
  - Input projections xi = W_ih @ x + b are computed in a separate first
    phase (big N=512 matmuls) and staged in DRAM in a chunk-local layout
    (warmup slivers duplicated) so both the proj writes and the scan's
    16-step window reads are large contiguous DMAs.
  - h never round-trips DRAM for the MLP: the scan writes h directly into
    a persistent SBUF tensor (t-major within chunks). For the begin/end
    gathers a token-major copy h_tok (fp32) is produced by PE transposes
    (128x128) DMA'd straight out of PSUM, overlapped with the scan.
  - Final phase: indirect row-gathers at begin/end boundaries -> local
    features -> section one-hot bmm -> 2-layer MLP, all tensor-engine
    friendly (N=512 moving tiles).
"""

import numpy as np
import ml_dtypes
from contextlib import ExitStack

import concourse.bass as bass
import concourse.tile as tile
from concourse import bacc
from concourse import mybir
from concourse.bass import ds
from concourse.bass_utils import run_bass_kernel_spmd

F32 = mybir.dt.float32
BF16 = mybir.dt.bfloat16
I32 = mybir.dt.int32
AF = mybir.ActivationFunctionType
OP = mybir.AluOpType

P = 128


class Cfg:
    def __init__(self, S=1024):
        assert S == 1024
        self.S = S
        self.B = 8            # batch per core
        self.I = 512
        self.H = 256
        self.G = 3 * self.H   # 768
        self.MLP = 512
        self.K = 64
        self.NT = self.S * self.B          # 8192 tokens per core
        self.nI = self.I // P              # 4
        self.nG = self.G // P              # 6
        self.nH = self.H // P              # 2
        # chunked scan
        self.C = 16                        # time chunks
        self.L = self.S // self.C          # 64 main steps per chunk
        self.W = 32                        # warmup steps
        self.D = self.L + self.W           # 96 scan steps per dir
        self.NB = self.C * self.B          # 128 scan columns
        self.WS = 16                       # steps per xi/h window
        self.NWIN = self.D // self.WS      # 6 windows (0,1 warmup; 2..5 main)
        self.XBLK = self.D * self.B        # 768 cols per chunk block in xiT
        self.XCOLS = self.C * self.XBLK    # 12288
        self.HTOK = self.NT + 8            # token-major h rows (+zero pad)


def build_program(cfg: Cfg):
    c = cfg
    nc = bacc.Bacc("TRN2", target_bir_lowering=False, debug=False)

    io = {}
    io["xT"] = nc.dram_tensor("xT", [c.I, c.NT], BF16, kind="ExternalInput").ap()
    for d in "fb":
        io[f"wihT_{d}"] = nc.dram_tensor(f"wihT_{d}", [c.I, c.G], BF16,
                                         kind="ExternalInput").ap()
        io[f"whhT_{d}"] = nc.dram_tensor(f"whhT_{d}", [c.H, c.G], BF16,
                                         kind="ExternalInput").ap()
        io[f"bxi_{d}"] = nc.dram_tensor(f"bxi_{d}", [P, c.nG], F32,
                                        kind="ExternalInput").ap()
        io[f"bhn_{d}"] = nc.dram_tensor(f"bhn_{d}", [P, c.nH], F32,
                                        kind="ExternalInput").ap()
        io[f"xiT_{d}"] = nc.dram_tensor(f"xiT_{d}", [c.G, c.XCOLS], BF16,
                                        kind="Internal").ap()
        io[f"h_tok_{d}"] = nc.dram_tensor(f"h_tok_{d}", [c.HTOK, c.H], BF16,
                                          kind="Internal").ap()
    io["ident"] = nc.dram_tensor("ident", [P, P], BF16, kind="ExternalInput").ap()
    io["w1T"] = nc.dram_tensor("w1T", [4 * c.H, c.MLP], BF16, kind="ExternalInput").ap()
    io["b1"] = nc.dram_tensor("b1", [P, c.MLP // P], F32, kind="ExternalInput").ap()
    io["w2T"] = nc.dram_tensor("w2T", [P, c.MLP // P], BF16, kind="ExternalInput").ap()
    io["b2v"] = nc.dram_tensor("b2v", [1, 1], F32, kind="ExternalInput").ap()
    io["secT"] = nc.dram_tensor("secT", [c.B, c.K, c.S], BF16,
                                kind="ExternalInput").ap()
    io["gidx"] = nc.dram_tensor("gidx", [4, c.B, c.K, 1], I32,
                                kind="ExternalInput").ap()
    io["out"] = nc.dram_tensor("out", [c.NT, 1], F32, kind="ExternalOutput").ap()

    with tile.TileContext(nc) as tc:
        _body(tc, c, io)
    nc.compile()
    return nc


def _body(tc, c, io):
    nc = tc.nc
    dirs = "fb"

    with ExitStack() as octx:
        # ---------------- persistent across phases ----------------
        wpool = octx.enter_context(tc.tile_pool(name="persist", bufs=1))
        whh_sb = {d: [wpool.tile([P, c.G], BF16, tag=f"whh{d}{ch}", name=f"whh{d}{ch}")
                      for ch in range(c.nH)] for d in dirs}
        bxi_sb = {d: wpool.tile([P, c.nG], F32, tag=f"bxi{d}", name=f"bxi{d}")
                  for d in dirs}
        bhn_sb = {d: wpool.tile([P, c.nH], F32, tag=f"bhn{d}", name=f"bhn{d}")
                  for d in dirs}
        ident_sb = wpool.tile([P, P], BF16, tag="ident", name="ident")
        b2_sb = wpool.tile([1, 1], F32, tag="b2", name="b2")
        # full hidden state, SBUF-resident: col = ch*8192 + t*8 + b
        h_sb = {d: wpool.tile([P, c.nH * c.NT], BF16, tag=f"hsb{d}",
                              name=f"hsb{d}") for d in dirs}

        for d in dirs:
            for ch in range(c.nH):
                nc.sync.dma_start(whh_sb[d][ch][:],
                                  io[f"whhT_{d}"][ch * P:(ch + 1) * P, :])
            nc.sync.dma_start(bxi_sb[d][:], io[f"bxi_{d}"][:])
            nc.sync.dma_start(bhn_sb[d][:], io[f"bhn_{d}"][:])
        nc.sync.dma_start(ident_sb[:], io["ident"][:])
        nc.sync.dma_start(b2_sb[:], io["b2v"][:])

        # zero pads: xiT warmup-of-nothing regions + h_tok pad rows
        zpool = octx.enter_context(tc.tile_pool(name="zeros", bufs=1))
        zx = zpool.tile([P, c.W * c.B], BF16, tag="zx", name="zx")  # [128,256]
        nc.vector.memset(zx[:], 0.0)
        zrow = zpool.tile([8, c.H], BF16, tag="zrow", name="zrow")
        nc.vector.memset(zrow[:], 0.0)
        xi_r = {d: io[f"xiT_{d}"].rearrange("(g p) (ch u) -> p g ch u",
                                            g=c.nG, ch=c.C) for d in dirs}
        # dir f: block 0 cols [0,256) are t<0 ; dir b: block 15 cols [512,768)
        for g in range(c.nG):
            nc.sync.dma_start(xi_r["f"][:, g, 0, 0:c.W * c.B], zx[:])
            nc.sync.dma_start(xi_r["b"][:, g, c.C - 1, c.L * c.B:c.XBLK], zx[:])
        for d in dirs:
            nc.sync.dma_start(io[f"h_tok_{d}"][c.NT:c.HTOK, :], zrow[:])

        # ================= phase 1: input projection =================
        with ExitStack() as ctx:
            wih_pool = ctx.enter_context(tc.tile_pool(name="wih", bufs=1))
            wih_sb = {d: [wih_pool.tile([P, c.G], BF16, tag=f"wih{d}{k}",
                                        name=f"wih{d}{k}")
                          for k in range(c.nI)] for d in dirs}
            for d in dirs:
                for k in range(c.nI):
                    nc.sync.dma_start(wih_sb[d][k][:],
                                      io[f"wihT_{d}"][k * P:(k + 1) * P, :])
            xpool = ctx.enter_context(tc.tile_pool(name="xtiles", bufs=2))
            xopool = ctx.enter_context(tc.tile_pool(name="xiout", bufs=3))
            pj_psum = ctx.enter_context(
                tc.tile_pool(name="pjps", bufs=1, space="PSUM"))

            TJ = 512  # token columns per tile = 64 t * 8 b = one chunk
            for j in range(c.NT // TJ):  # 16 tiles == 16 chunks
                xk = [xpool.tile([P, TJ], BF16, tag=f"x{k}", name=f"x{k}")
                      for k in range(c.nI)]
                for k in range(c.nI):
                    nc.sync.dma_start(xk[k][:],
                                      io["xT"][k * P:(k + 1) * P,
                                               j * TJ:(j + 1) * TJ])
                for d in dirs:
                    ps = [pj_psum.tile([P, TJ], F32, tag=f"pj{g}", name=f"pj{g}")
                          for g in range(c.nG)]
                    for g in range(c.nG):
                        for k in range(c.nI):
                            nc.tensor.matmul(ps[g][:],
                                             wih_sb[d][k][:, g * P:(g + 1) * P],
                                             xk[k][:],
                                             start=(k == 0), stop=(k == c.nI - 1))
                    xo = xopool.tile([P, c.nG * TJ], BF16, tag=f"xo{d}",
                                     name=f"xo{d}")
                    xo_r = xo[:].rearrange("p (g u) -> p g u", g=c.nG)
                    for g in range(c.nG):
                        if g % 2 == 0:
                            nc.scalar.activation(xo_r[:, g, :], ps[g][:],
                                                 AF.Identity,
                                                 bias=bxi_sb[d][:, g:g + 1])
                        else:
                            nc.vector.tensor_scalar_add(xo_r[:, g, :], ps[g][:],
                                                        bxi_sb[d][:, g:g + 1])
                    # main write: tile j == chunk j's main tokens
                    if d == "f":
                        nc.sync.dma_start(
                            xi_r[d][:, :, j, c.W * c.B:c.XBLK], xo_r)
                        if j < c.C - 1:  # dup: last 32 t -> chunk j+1 warmup
                            nc.scalar.dma_start(
                                xi_r[d][:, :, j + 1, 0:c.W * c.B],
                                xo_r[:, :, c.L // 2 * c.B:])
                    else:
                        nc.sync.dma_start(
                            xi_r[d][:, :, j, 0:c.L * c.B], xo_r)
                        if j > 0:  # dup: first 32 t -> chunk j-1 warmup
                            nc.scalar.dma_start(
                                xi_r[d][:, :, j - 1, c.L * c.B:c.XBLK],
                                xo_r[:, :, 0:c.W * c.B])

        tc.strict_bb_all_engine_barrier()

        # ================= phase 2: chunked scan =================
        with ExitStack() as ctx:
            xiw_pool = ctx.enter_context(tc.tile_pool(name="xiw", bufs=2))
            warm_pool = ctx.enter_context(tc.tile_pool(name="warm", bufs=2))
            z0_pool = ctx.enter_context(tc.tile_pool(name="z0", bufs=1))
            gpool = ctx.enter_context(tc.tile_pool(name="gates", bufs=2))
            gh_psum = ctx.enter_context(
                tc.tile_pool(name="ghps", bufs=1, space="PSUM"))
            tr_psum = ctx.enter_context(
                tc.tile_pool(name="trps", bufs=2, space="PSUM"))

            WCOLS = c.nG * c.C * c.WS * c.B  # 12288 cols per window tile

            zwarm = {d: z0_pool.tile([P, c.nH * c.NB], BF16, tag=f"zw{d}",
                                     name=f"zw{d}") for d in dirs}
            for d in dirs:
                nc.vector.memset(zwarm[d][:], 0.0)

            win_tile = {d: None for d in dirs}
            prev_state = {d: zwarm[d] for d in dirs}  # tile holding h(s-1)
            prev_is_warm = {d: True for d in dirs}

            def issue_window(d, w):
                t_ = xiw_pool.tile([P, WCOLS], BF16, tag=f"xiw{d}",
                                   name=f"xiw{d}")
                # src: per chunk block, 128-step-contig slice
                if d == "f":
                    u0 = w * c.WS * c.B
                else:
                    u0 = (c.NWIN - 1 - w) * c.WS * c.B
                dst = t_[:].rearrange("p (g ch u) -> p g ch u", g=c.nG, ch=c.C)
                for g in range(c.nG):
                    nc.sync.dma_start(dst[:, g],
                                      xi_r[d][:, g, :, u0:u0 + c.WS * c.B])
                return t_

            # h_sb view helpers
            hview = {d: h_sb[d][:].rearrange("p (ch c q b) -> p ch c q b",
                                             ch=c.nH, c=c.C, q=c.L)
                     for d in dirs}

            def step(d, s):
                w, dt = divmod(s, c.WS)
                xslot = dt if d == "f" else c.WS - 1 - dt
                q = (s - c.W) if d == "f" else (c.D - 1 - s)
                xir = win_tile[d][:].rearrange("p (g ch t b) -> p g ch t b",
                                               g=c.nG, ch=c.C, t=c.WS)

                # previous state views: [p, ch, c, b]
                if prev_is_warm[d]:
                    pv = prev_state[d][:].rearrange("p (ch c b) -> p ch c b",
                                                    ch=c.nH, c=c.C)
                else:
                    qp = q + 1 if d == "b" else q - 1
                    pv = hview[d][:, :, :, qp, :]

                gh = gh_psum.tile([P, c.G], F32, tag=f"gh{d}", name=f"gh{d}")
                for g in range(c.nG):
                    for ch in range(c.nH):
                        nc.tensor.matmul(gh[:, g * P:(g + 1) * P],
                                         whh_sb[d][ch][:, g * P:(g + 1) * P],
                                         pv[:, ch, :, :],
                                         start=(ch == 0), stop=(ch == c.nH - 1))
                ghr = gh[:].rearrange("p (g c b) -> p g c b", g=c.nG, c=c.C)

                rz = gpool.tile([P, 4 * c.NB], BF16, tag=f"rz{d}", name=f"rz{d}")
                rzr = rz[:].rearrange("p (g c b) -> p g c b", g=4, c=c.C)
                nc.vector.tensor_tensor(rzr[:, 0:2], ghr[:, 0:2],
                                        xir[:, 0:2, :, xslot, :], OP.add)
                nc.vector.tensor_tensor(rzr[:, 2:4], ghr[:, 2:4],
                                        xir[:, 2:4, :, xslot, :], OP.add)
                sg = gpool.tile([P, 4 * c.NB], BF16, tag=f"sg{d}", name=f"sg{d}")
                nc.scalar.activation(sg[:], rz[:], AF.Sigmoid)

                t1 = gpool.tile([P, c.nH * c.NB], BF16, tag=f"t1{d}", name=f"t1{d}")
                for ch in range(c.nH):
                    nc.vector.scalar_tensor_tensor(
                        t1[:, ch * c.NB:(ch + 1) * c.NB],
                        gh[:, 4 * P + ch * c.NB:4 * P + (ch + 1) * c.NB],
                        bhn_sb[d][:, ch:ch + 1],
                        sg[:, ch * c.NB:(ch + 1) * c.NB],
                        OP.add, OP.mult)
                t2 = gpool.tile([P, c.nH * c.NB], BF16, tag=f"t2{d}", name=f"t2{d}")
                nc.gpsimd.tensor_tensor(
                    t2[:].rearrange("p (g c b) -> p g c b", g=c.nH, c=c.C),
                    t1[:].rearrange("p (g c b) -> p g c b", g=c.nH, c=c.C),
                    xir[:, 4:6, :, xslot, :], OP.add)
                n_ = gpool.tile([P, c.nH * c.NB], BF16, tag=f"n{d}", name=f"n{d}")
                nc.scalar.activation(n_[:], t2[:], AF.Tanh)

                nv = n_[:].rearrange("p (ch c b) -> p ch c b", ch=c.nH, c=c.C)
                d_ = gpool.tile([P, c.nH * c.NB], BF16, tag=f"dd{d}", name=f"dd{d}")
                dv = d_[:].rearrange("p (ch c b) -> p ch c b", ch=c.nH, c=c.C)
                nc.gpsimd.tensor_tensor(dv, pv, nv, OP.subtract)
                e_ = gpool.tile([P, c.nH * c.NB], BF16, tag=f"ee{d}", name=f"ee{d}")
                nc.gpsimd.tensor_tensor(e_[:], sg[:, 2 * c.NB:4 * c.NB], d_[:],
                                        OP.mult)
                ev = e_[:].rearrange("p (ch c b) -> p ch c b", ch=c.nH, c=c.C)

                if s < c.W:
                    out_t = warm_pool.tile([P, c.nH * c.NB], BF16,
                                           tag=f"warm{d}", name=f"warm{d}")
                    ov = out_t[:].rearrange("p (ch c b) -> p ch c b",
                                            ch=c.nH, c=c.C)
                    nc.vector.tensor_tensor(ov, nv, ev, OP.add)
                    prev_state[d] = out_t
                    prev_is_warm[d] = True
                else:
                    ov = hview[d][:, :, :, q, :]
                    nc.vector.tensor_tensor(ov, nv, ev, OP.add)
                    prev_is_warm[d] = False

            # prologue: windows 0 and 1 in flight
            win_tile = {d: issue_window(d, 0) for d in dirs}
            pending_win = {d: issue_window(d, 1) for d in dirs}

            for s in range(c.D):
                if s % c.WS == 0 and s > 0:
                    win_tile = dict(pending_win)
                    w_next = s // c.WS + 1
                    if w_next < c.NWIN:
                        pending_win = {d: issue_window(d, w_next) for d in dirs}
                for d in dirs:
                    step(d, s)
                if s == c.W - 1:
                    # chunk 0 (fwd) / chunk 15 (bwd) start from true h0 = 0
                    pf = prev_state["f"][:].rearrange("p (ch c b) -> p ch c b",
                                                      ch=c.nH, c=c.C)
                    nc.vector.memset(pf[:, :, 0, :], 0.0)
                    pb = prev_state["b"][:].rearrange("p (ch c b) -> p ch c b",
                                                      ch=c.nH, c=c.C)
                    nc.vector.memset(pb[:, :, c.C - 1, :], 0.0)
                # end-of-main-window: transpose h block -> h_tok (fp32)
                if s % c.WS == c.WS - 1 and s >= c.W:
                    w = s // c.WS
                    for d in dirs:
                        qb = (w - 2) * c.WS if d == "f" else (c.NWIN - 1 - w) * c.WS
                        for ch in range(c.nH):
                            for cc in range(c.C):
                                colb = ch * c.NT + cc * c.L * c.B + qb * c.B
                                trp = tr_psum.tile([P, P], BF16, tag="tr",
                                                   name="tr")
                                nc.tensor.transpose(
                                    trp[:], h_sb[d][:, colb:colb + P],
                                    ident_sb[:])
                                trs = gpool.tile([P, P], BF16, tag="trs",
                                                 name="trs")
                                nc.vector.tensor_copy(trs[:], trp[:])
                                rowb = (cc * c.L + qb) * c.B
                                nc.sync.dma_start(
                                    io[f"h_tok_{d}"][rowb:rowb + P,
                                                     ch * P:(ch + 1) * P],
                                    trs[:])

        tc.strict_bb_all_engine_barrier()

        # ============ phase 3: gathers + local + bmm + MLP ============
        with ExitStack() as ctx:
            lpool = ctx.enter_context(tc.tile_pool(name="loc", bufs=1))
            gxpool = ctx.enter_context(tc.tile_pool(name="gx", bufs=4))
            local = [lpool.tile([c.K, 2 * c.H], BF16, tag=f"loc{b}",
                                name=f"loc{b}") for b in range(c.B)]
            for b in range(c.B):
                g = {}
                for gi, (nm, dd) in enumerate(
                        (("fe", "f"), ("fb", "f"), ("bb", "b"), ("be", "b"))):
                    idx = gxpool.tile([c.K, 1], I32, tag="gidx", name="gidx")
                    nc.sync.dma_start(idx[:], io["gidx"][gi, b, :, :])
                    gt = gxpool.tile([c.K, c.H], BF16, tag=f"g{gi}", name=f"g{gi}")
                    nc.gpsimd.indirect_dma_start(
                        out=gt[:], out_offset=None, in_=io[f"h_tok_{dd}"][:],
                        in_offset=bass.IndirectOffsetOnAxis(ap=idx[:, :1], axis=0),
                        bounds_check=c.HTOK - 1, oob_is_err=False)
                    g[nm] = gt
                nc.vector.tensor_tensor(local[b][:, 0:c.H], g["fe"][:],
                                        g["fb"][:], OP.subtract)
                nc.vector.tensor_tensor(local[b][:, c.H:], g["bb"][:],
                                        g["be"][:], OP.subtract)

            w1pool = ctx.enter_context(tc.tile_pool(name="w1", bufs=1))
            nMI = 4 * c.H // P  # 8
            w1_sb = [w1pool.tile([P, c.MLP], BF16, tag=f"w1_{i}", name=f"w1_{i}")
                     for i in range(nMI)]
            for i in range(nMI):
                nc.sync.dma_start(w1_sb[i][:], io["w1T"][i * P:(i + 1) * P, :])
            b1_sb = w1pool.tile([P, c.MLP // P], F32, tag="b1", name="b1")
            nc.sync.dma_start(b1_sb[:], io["b1"][:])
            w2_sb = w1pool.tile([P, c.MLP // P], BF16, tag="w2", name="w2")
            nc.sync.dma_start(w2_sb[:], io["w2T"][:])
            secpool = ctx.enter_context(tc.tile_pool(name="sec", bufs=1))
            sec_sb = [secpool.tile([c.K, c.S], BF16, tag=f"sec{b}", name=f"sec{b}")
                      for b in range(c.B)]
            for b in range(c.B):
                nc.sync.dma_start(sec_sb[b][:], io["secT"][b, :, :])

            # lcr[fc] : [128, (t 1024)(b 8)] bf16, all tokens
            lcrpool = ctx.enter_context(tc.tile_pool(name="lcr", bufs=1))
            nLC = 2 * c.H // P  # 4
            lcr_sb = [lcrpool.tile([P, c.NT], BF16, tag=f"lcr{fc}",
                                   name=f"lcr{fc}") for fc in range(nLC)]
            l_psum = ctx.enter_context(
                tc.tile_pool(name="lps", bufs=2, space="PSUM"))
            for hf in range(2):
                for b in range(c.B):
                    for fc in range(nLC):
                        lps = l_psum.tile([P, c.S // 2], F32, tag="lps",
                                          name="lps")
                        nc.tensor.matmul(
                            lps[:], local[b][:, fc * P:(fc + 1) * P],
                            sec_sb[b][:, hf * 512:(hf + 1) * 512],
                            start=True, stop=True)
                        dst = lcr_sb[fc][:].rearrange(
                            "p (t b) -> p t b", b=c.B)[:, hf * 512:(hf + 1) * 512, b]
                        if (b + fc) % 2 == 0:
                            nc.scalar.activation(dst, lps[:], AF.Copy)
                        else:
                            nc.vector.tensor_copy(dst, lps[:])

            # MLP over 16 token tiles of 512 (t,b)-cols
            mpool = ctx.enter_context(tc.tile_pool(name="mlp", bufs=2))
            h1_psum = ctx.enter_context(
                tc.tile_pool(name="h1ps", bufs=2, space="PSUM"))
            o_psum = ctx.enter_context(
                tc.tile_pool(name="ops", bufs=2, space="PSUM"))
            TT = 512
            for j in range(c.NT // TT):
                rhs = []
                for d in dirs:
                    for ch in range(c.nH):
                        rhs.append(h_sb[d][:, ch * c.NT + j * TT:
                                           ch * c.NT + (j + 1) * TT])
                for fc in range(nLC):
                    rhs.append(lcr_sb[fc][:, j * TT:(j + 1) * TT])
                h1t = []
                for mc in range(c.MLP // P):
                    hps = h1_psum.tile([P, TT], F32, tag="h1ps", name="h1ps")
                    for ic in range(nMI):
                        nc.tensor.matmul(hps[:],
                                         w1_sb[ic][:, mc * P:(mc + 1) * P],
                                         rhs[ic], start=(ic == 0),
                                         stop=(ic == nMI - 1))
                    h1 = mpool.tile([P, TT], BF16, tag=f"h1_{mc}",
                                    name=f"h1_{mc}")
                    nc.scalar.activation(h1[:], hps[:], AF.Relu,
                                         bias=b1_sb[:, mc:mc + 1])
                    h1t.append(h1)
                pso = o_psum.tile([1, TT], F32, tag="pso", name="pso")
                for mc in range(c.MLP // P):
                    nc.tensor.matmul(pso[:], w2_sb[:, mc:mc + 1], h1t[mc][:],
                                     start=(mc == 0), stop=(mc == c.MLP // P - 1))
                ot = mpool.tile([1, TT], F32, tag="ot", name="ot")
                nc.scalar.activation(ot[:], pso[:], AF.Identity,
                                     bias=b2_sb[0:1, 0:1])
                nc.sync.dma_start(io["out"][j * TT:(j + 1) * TT, :], ot[:])


# ======================= host side =======================

def _prep_core(inputs_np, core, c):
    bf = ml_dtypes.bfloat16
    bsl = slice(core * c.B, (core + 1) * c.B)
    x = inputs_np["inputs"][:, bsl, :]
    feed = {}
    feed["xT"] = np.ascontiguousarray(
        x.transpose(2, 0, 1).reshape(c.I, c.NT)).astype(bf)
    for d, sfx in (("f", "_f"), ("b", "_b")):
        wih = inputs_np["W_ih" + sfx]
        whh = inputs_np["W_hh" + sfx]
        bih = inputs_np["b_ih" + sfx].astype(np.float32)
        bhh = inputs_np["b_hh" + sfx].astype(np.float32)
        feed[f"wihT_{d}"] = np.ascontiguousarray(wih.T).astype(bf)
        feed[f"whhT_{d}"] = np.ascontiguousarray(whh.T).astype(bf)
        bxi = bih + np.concatenate([bhh[:2 * c.H], np.zeros(c.H, np.float32)])
        feed[f"bxi_{d}"] = np.ascontiguousarray(bxi.reshape(c.nG, P).T)
        feed[f"bhn_{d}"] = np.ascontiguousarray(bhh[2 * c.H:].reshape(c.nH, P).T)
    feed["ident"] = np.eye(P, dtype=np.float32).astype(bf)
    feed["w1T"] = np.ascontiguousarray(inputs_np["W1"].T).astype(bf)
    feed["b1"] = np.ascontiguousarray(
        inputs_np["b1"].astype(np.float32).reshape(c.MLP // P, P).T)
    feed["w2T"] = np.ascontiguousarray(
        inputs_np["W2"].reshape(c.MLP).reshape(c.MLP // P, P).T).astype(bf)
    feed["b2v"] = np.array([[float(np.asarray(inputs_np["b2"]).reshape(-1)[0])]],
                           np.float32)
    feed["secT"] = np.ascontiguousarray(
        inputs_np["section_indicator"][bsl].transpose(0, 2, 1)).astype(bf)
    beg = np.asarray(inputs_np["begin"][bsl]).astype(np.int64)
    end = np.asarray(inputs_np["end"][bsl]).astype(np.int64)
    BIG = c.NT
    bvec = np.arange(c.B)[:, None]

    def rows(v):
        return np.where(v > 0, (v - 1) * c.B + bvec, BIG).astype(np.int32)

    gi = np.stack([rows(end), rows(beg), rows(beg), rows(end)])  # fe, fb, bb, be
    feed["gidx"] = np.ascontiguousarray(gi.reshape(4, c.B, c.K, 1))
    return feed


_PROG_CACHE = {}
LAST_RESULTS = None


def _get_prog(c: Cfg):
    if c.S not in _PROG_CACHE:
        _PROG_CACHE[c.S] = build_program(c)
    return _PROG_CACHE[c.S]


def kernel(**inputs):
    c = Cfg(S=np.asarray(inputs["inputs"]).shape[0])
    inputs_np = {k: np.asarray(v) for k, v in inputs.items()}
    global LAST_RESULTS
    nc = _get_prog(c)
    in_maps = [_prep_core(inputs_np, core, c) for core in range(8)]
    res = run_bass_kernel_spmd(nc, in_maps, core_ids=list(range(8)))
    LAST_RESULTS = res
    outs = [res.results[core]["out"].reshape(c.S, c.B, 1) for core in range(8)]
    return np.concatenate(outs, axis=1).astype(np.float32)
